# revision 20
# baseline (speedup 1.0000x reference)
"""CrossLinearAttention Trainium2 kernel (8 NeuronCores, SPMD).

Problem: b=4, n1=n2=8192, dim=256, 8 heads x 64 dim_head.
  q = x @ Wq.T                     (rotary 2D on q)
  k, v = split(z @ Wkv.T)          (LayerNorm per head-dim on k, v; rotary on k)
  dots = k^T v / n2 per (b, h);  out = (q @ dots) @ Wo.T + bo

Sharding: flatten (b, n) -> 32768 rows; core c owns rows [4096c, 4096(c+1))
(one half of batch c//2).  The z-side partial k^T v is summed with a
pairwise AllReduce {0,1},{2,3},{4,5},{6,7}.

Design (v2):
  - Host pre-transposes x/z and converts inputs+weights+tables to fp16,
    so the kernel never runs PE transposes and all elementwise work runs
    in the DVE 2-byte fast path.
  - x side computes qT = Wq @ xT directly in transposed layout; the
    rotate-half term is a second projection with row-permuted Wq, so
    rotary is pure elementwise with partition-replicated tables.
  - z side is row-major (lhsT = zT chunks).  Wkv is host-centered (+
    gamma folded), so LayerNorm is sumsq -> rstd; rstd_k*rstd_v folds
    into v (trivial-affine case).  k rotary uses sign-baked sin tables
    and a stride trick for rotate-half.
  - kv / q PSUM results are copied once to SBUF fp16 by the scalar
    engine; every downstream elementwise op is then 2-byte packed SBUF
    (DVE 2x mode).  The LN reduce is a 2-level pairwise fold + short
    reduce.
  - dots accumulates transposed (lhsT = v2) into PSUM across all 32 row
    tiles; x-side work lags the z side by 16 iterations so the pairwise
    AllReduce overlaps the x tail (which avoids the gpsimd queue, where
    the collective is triggered).  dots is folded into Wo
    (W2 = blkdiag(dotsT) @ WoT * 1/n2), so the output stage is
    y = rotqT.T @ W2 + bo.
"""

import sys

sys.path.insert(0, "/opt/trn_rl_repo")

from contextlib import ExitStack

import numpy as np

import concourse.bass as bass
import concourse.tile as tile
from concourse import bacc, mybir
from concourse.bass import ts
from concourse.bass_utils import run_bass_kernel_spmd

F32 = mybir.dt.float32
F16 = mybir.dt.float16
ALU = mybir.AluOpType
AX = mybir.AxisListType
AF = mybir.ActivationFunctionType

B, N1, DIM = 4, 8192, 256
H, DH = 8, 64
INNER = H * DH  # 512
NCORES = 8
ROWS = (B * N1) // NCORES  # 4096 rows per core
NT = ROWS // 128  # 32 tiles of 128 rows
NG = ROWS // 512  # 8 x-side groups of 512 rows
LAG = 16  # x-side iteration lag behind z side
EPS = 1e-5

_nc_cache = {}


def _view(ap, offset, dims):
    """AP view with explicit free dims; keeps the partition dim."""
    return bass.AP(
        tensor=ap.tensor,
        offset=ap.offset + offset,
        ap=[list(ap.ap[0])] + [list(d) for d in dims],
    )


def build_nc(triv: bool):
    if triv in _nc_cache:
        return _nc_cache[triv]
    nc = bacc.Bacc(trn_type="TRN2", num_devices=NCORES, debug=False)

    zt_d = nc.dram_tensor("zt", [DIM, ROWS], F16, kind="ExternalInput").ap()
    xt_d = nc.dram_tensor("xt", [DIM, ROWS], F16, kind="ExternalInput").ap()
    wq_d = nc.dram_tensor("wq", [DIM, INNER], F16, kind="ExternalInput").ap()
    wqrh_d = nc.dram_tensor("wqrh", [DIM, INNER], F16, kind="ExternalInput").ap()
    wkv_d = nc.dram_tensor("wkv", [DIM, 2 * INNER], F16, kind="ExternalInput").ap()
    wo_d = nc.dram_tensor("wo", [INNER, DIM], F16, kind="ExternalInput").ap()
    bo_d = nc.dram_tensor("bo", [1, 2 * DIM], F32, kind="ExternalInput").ap()
    # k tables, row-major: [rows, 128] = cos(64, gamma_k folded) ||
    # sign-baked sin(64, gamma_k at swapped index folded)
    ktab_d = nc.dram_tensor("ktab", [ROWS, 128], F16, kind="ExternalInput").ap()
    # q tables, partition-replicated: [128, 2*rows] = cos || sign-sin
    qtab_d = nc.dram_tensor("qtab", [128, 2 * ROWS], F16, kind="ExternalInput").ap()
    if not triv:
        rbk_d = nc.dram_tensor("rbk", [ROWS, 64], F16, kind="ExternalInput").ap()
        bv_d = nc.dram_tensor("bv", [1, INNER], F32, kind="ExternalInput").ap()
    y_d = nc.dram_tensor("y", [ROWS, DIM], F32, kind="ExternalOutput").ap()
    cc_in = nc.dram_tensor("cc_in", [128, 512], F16)
    cc_out = nc.dram_tensor("cc_out", [128, 512], F16)

    with tile.TileContext(nc) as tc, ExitStack() as ctx:
        consts = ctx.enter_context(tc.tile_pool(name="consts", bufs=1))

        eps_t = consts.tile([128, 1], F32)
        nc.vector.memset(eps_t, EPS)

        # wkv is needed by the very first matmul; every other weight DMA is
        # deferred and dribbled out one-per-iteration so the z-side input
        # DMAs are not stuck behind them in the sync queue.
        wkv_t = []
        for kc in range(2):
            t = consts.tile([128, 2 * INNER], F16, tag=f"wkv{kc}")
            nc.sync.dma_start(t, wkv_d[ts(kc, 128), :])
            wkv_t.append(t)
        deferred = []
        wq_t = []
        wqrh_t = []
        for kc in range(2):
            t = consts.tile([128, INNER], F16, tag=f"wq{kc}")
            deferred.append((t, wq_d[ts(kc, 128), :]))
            wq_t.append(t)
            t = consts.tile([128, INNER], F16, tag=f"wqrh{kc}")
            deferred.append((t, wqrh_d[ts(kc, 128), :]))
            wqrh_t.append(t)
        wo_t = []
        for p in range(4):
            t = consts.tile([128, DIM], F16, tag=f"wo{p}")
            deferred.append((t, wo_d[ts(p, 128), :]))
            wo_t.append(t)
        bo_bc = consts.tile([128, 2 * DIM], F32)
        deferred.append(
            (bo_bc, _view(bo_d, 0, [[1, 2 * DIM]]).partition_broadcast(128))
        )
        if not triv:
            bv_bc = consts.tile([128, INNER], F32)
            deferred.append(
                (bv_bc, _view(bv_d, 0, [[1, INNER]]).partition_broadcast(128))
            )

        rotq_sb = []
        for c in range(4):
            rotq_c = consts.tile([128, ROWS], F16, tag=f"rotq{c}", name=f"rotq{c}")
            rotq_sb.append(rotq_c)

        dots_sb = consts.tile([128, 512], F16)
        dots_rd = consts.tile([128, 512], F16)
        blk = consts.tile([128, 512], F16)
        w2_sb = consts.tile([128, 1024], F16)

        def x_iter(j, zin, elem, stats, q_pp, use_gpsimd):
            """One x-side iteration: group g = j//4, inner chunk c = j%4."""
            g, c = j // 4, j % 4
            if c == 0:
                xc = []
                for kc in range(2):
                    t = zin.tile([128, 512], F16, tag=f"xc{kc}", name=f"xc{kc}")
                    nc.sync.dma_start(t, xt_d[ts(kc, 128), ts(g, 512)])
                    xc.append(t)
                qcos = zin.tile([128, 512], F16, tag="qcos")
                nc.sync.dma_start(qcos, qtab_d[:, ts(g, 512)])
                qsin = zin.tile([128, 512], F16, tag="qsin")
                nc.sync.dma_start(
                    qsin, qtab_d[:, ROWS + g * 512 : ROWS + (g + 1) * 512]
                )
                x_iter.cur = (xc, qcos, qsin)
            xc, qcos, qsin = x_iter.cur

            q_ps = q_pp.tile([128, 1024], F32, tag="q_ps")
            for kc in range(2):
                nc.tensor.matmul(
                    q_ps[:, 0:512],
                    wq_t[kc][:, ts(c, 128)],
                    xc[kc],
                    start=(kc == 0),
                    stop=(kc == 1),
                )
            for kc in range(2):
                nc.tensor.matmul(
                    q_ps[:, 512:1024],
                    wqrh_t[kc][:, ts(c, 128)],
                    xc[kc],
                    start=(kc == 0),
                    stop=(kc == 1),
                )
            q16 = elem.tile([128, 1024], F16, tag="q16")
            nc.scalar.copy(q16, q_ps)
            qcs = elem.tile([128, 512], F16, tag="qcs")
            qrs = elem.tile([128, 512], F16, tag="qrs")
            if use_gpsimd:
                # z-loop iterations: gpsimd takes both table mults
                nc.gpsimd.tensor_tensor(qcs, q16[:, 0:512], qcos, op=ALU.mult)
                nc.gpsimd.tensor_tensor(qrs, q16[:, 512:1024], qsin, op=ALU.mult)
            else:
                # AllReduce tail: the collective blocks the gpsimd queue
                nc.vector.tensor_tensor(qcs, q16[:, 0:512], qcos, op=ALU.mult)
                nc.vector.tensor_tensor(qrs, q16[:, 512:1024], qsin, op=ALU.mult)
            nc.vector.tensor_tensor(
                rotq_sb[c][:, ts(g, 512)], qcs, qrs, op=ALU.add
            )

        # ------------- main loop: z side, x side lagging by LAG -------------
        with ExitStack() as pz:
            zin = pz.enter_context(tc.tile_pool(name="zin", bufs=2))
            tabs = pz.enter_context(tc.tile_pool(name="tabs", bufs=3))
            elem = pz.enter_context(tc.tile_pool(name="elem", bufs=2))
            stats = pz.enter_context(tc.tile_pool(name="stats", bufs=3))
            kv_pp = pz.enter_context(tc.tile_pool(name="kv_pp", bufs=1, space="PSUM"))
            q_pp = pz.enter_context(tc.tile_pool(name="q_pp", bufs=1, space="PSUM"))
            dots_pp = pz.enter_context(
                tc.tile_pool(name="dots_pp", bufs=1, space="PSUM")
            )

            dots_t = []
            for p in range(4):
                dots_t.append(
                    dots_pp.tile([128, 256], F32, tag=f"dots{p}", name=f"dots{p}")
                )

            prev = None  # (krot, v2) awaiting dots matmuls
            for i in range(NT):
                if i % 4 == 0:
                    zc = []
                    for kc in range(2):
                        t = zin.tile([128, 512], F16, tag=f"zc{kc}", name=f"zc{kc}")
                        nc.sync.dma_start(t, zt_d[ts(kc, 128), ts(i // 4, 512)])
                        zc.append(t)
                ktab_t = tabs.tile([128, 128], F16, tag="ktab")
                nc.sync.dma_start(ktab_t, ktab_d[ts(i, 128), :])
                if not triv:
                    rbk_t = tabs.tile([128, 64], F16, tag="rbk")
                    nc.sync.dma_start(rbk_t, rbk_d[ts(i, 128), :])

                if i >= 1 and deferred:
                    dt_, dsrc = deferred.pop(0)
                    nc.sync.dma_start(dt_, dsrc)

                kv_ps = kv_pp.tile([128, 1024], F32, tag="kv_ps")
                for kc in range(2):
                    nc.tensor.matmul(
                        kv_ps[:, 0:512],
                        zc[kc][:, ts(i % 4, 128)],
                        wkv_t[kc][:, 0:INNER],
                        start=(kc == 0),
                        stop=(kc == 1),
                    )
                for kc in range(2):
                    nc.tensor.matmul(
                        kv_ps[:, 512:1024],
                        zc[kc][:, ts(i % 4, 128)],
                        wkv_t[kc][:, INNER : 2 * INNER],
                        start=(kc == 0),
                        stop=(kc == 1),
                    )

                # lagged x-side iteration (keeps PE busy while DVE catches up)
                if i >= LAG:
                    x_iter(i - LAG, zin, elem, stats, q_pp, use_gpsimd=True)

                # dots for the previous tile (gives DVE a full iteration)
                if prev is not None:
                    pk, pv = prev
                    for p in range(4):
                        nc.tensor.matmul(
                            dots_t[p],
                            pv[:, ts(p, 128)],
                            pk[:, (p // 2) * 256 : (p // 2) * 256 + 256],
                            start=(i == 1),
                            stop=False,
                        )

                kv16 = elem.tile([128, 2 * INNER], F16, tag="kv16")
                nc.scalar.copy(kv16, kv_ps)

                sq = elem.tile([128, 2 * INNER], F16, tag="sq")
                nc.scalar.activation(sq[:, 0:INNER], kv16[:, 0:INNER], AF.Square)
                nc.vector.tensor_tensor(
                    sq[:, INNER : 2 * INNER],
                    kv16[:, INNER : 2 * INNER],
                    kv16[:, INNER : 2 * INNER],
                    op=ALU.mult,
                )
                f1 = elem.tile([128, INNER], F16, tag="f1")
                nc.vector.tensor_tensor(
                    _view(f1, 0, [[32, 16], [1, 32]]),
                    _view(sq, 0, [[64, 16], [1, 32]]),
                    _view(sq, 32, [[64, 16], [1, 32]]),
                    op=ALU.add,
                )
                f2 = elem.tile([128, 256], F16, tag="f2")
                nc.vector.tensor_tensor(
                    _view(f2, 0, [[16, 16], [1, 16]]),
                    _view(f1, 0, [[32, 16], [1, 16]]),
                    _view(f1, 16, [[32, 16], [1, 16]]),
                    op=ALU.add,
                )
                sums = stats.tile([128, 16], F32, tag="sums")
                nc.vector.reduce_sum(
                    sums, _view(f2, 0, [[16, 16], [1, 16]]), axis=AX.X
                )
                std = stats.tile([128, 16], F32, tag="std")
                nc.scalar.activation(
                    std, sums, AF.Sqrt, scale=1.0 / DH, bias=eps_t[:, 0:1]
                )
                rstd = stats.tile([128, 16], F32, tag="rstd")
                nc.vector.reciprocal(rstd, std)

                # rotary on k (fp16 fast path): kcs = k*cos, krs = swap(k)*ssin
                kcs = elem.tile([128, INNER], F16, tag="kcs")
                krs = elem.tile([128, INNER], F16, tag="krs")
                krot = elem.tile([128, INNER], F16, tag="krot")
                v2 = elem.tile([128, INNER], F16, tag="v2")
                nc.vector.tensor_tensor(
                    _view(kcs, 0, [[64, 8], [32, 2], [16, 2], [1, 16]]),
                    _view(kv16, 0, [[64, 8], [32, 2], [16, 2], [1, 16]]),
                    _view(ktab_t, 0, [[0, 8], [32, 2], [16, 2], [1, 16]]),
                    op=ALU.mult,
                )
                nc.vector.tensor_tensor(
                    _view(krs, 0, [[64, 8], [32, 2], [16, 2], [1, 16]]),
                    _view(kv16, 16, [[64, 8], [32, 2], [-16, 2], [1, 16]]),
                    _view(ktab_t, 64, [[0, 8], [32, 2], [16, 2], [1, 16]]),
                    op=ALU.mult,
                )
                if triv:
                    rkv = stats.tile([128, 8], F32, tag="rkv")
                    nc.vector.tensor_tensor(
                        rkv, rstd[:, 0:8], rstd[:, 8:16], op=ALU.mult
                    )
                    nc.gpsimd.tensor_tensor(
                        _view(v2, 0, [[64, 8], [1, 64]]),
                        _view(kv16, 512, [[64, 8], [1, 64]]),
                        _view(rkv, 0, [[1, 8], [0, 64]]),
                        op=ALU.mult,
                    )
                    nc.vector.tensor_tensor(krot, kcs, krs, op=ALU.add)
                    prev = (krot, v2)
                else:
                    # krot = rstd_k * (kcs + krs) + rot(beta_k)
                    t1 = elem.tile([128, INNER], F16, tag="t1")
                    nc.vector.tensor_tensor(t1, kcs, krs, op=ALU.add)
                    t2 = elem.tile([128, INNER], F16, tag="t2")
                    nc.vector.tensor_tensor(
                        _view(t2, 0, [[64, 8], [1, 64]]),
                        _view(t1, 0, [[64, 8], [1, 64]]),
                        _view(rstd, 0, [[1, 8], [0, 64]]),
                        op=ALU.mult,
                    )
                    nc.gpsimd.tensor_tensor(
                        _view(krot, 0, [[64, 8], [1, 64]]),
                        _view(t2, 0, [[64, 8], [1, 64]]),
                        _view(rbk_t, 0, [[0, 8], [1, 64]]),
                        op=ALU.add,
                    )
                    # v2 = rstd_v * v + beta_v (gamma_v folded into Wkv)
                    t3 = elem.tile([128, INNER], F16, tag="t3")
                    nc.gpsimd.tensor_tensor(
                        _view(t3, 0, [[64, 8], [1, 64]]),
                        _view(kv16, 512, [[64, 8], [1, 64]]),
                        _view(rstd, 8, [[1, 8], [0, 64]]),
                        op=ALU.mult,
                    )
                    v2b = elem.tile([128, INNER], F16, tag="v2b")
                    nc.vector.tensor_tensor(v2b, t3, bv_bc, op=ALU.add)
                    prev = (krot, v2b)

            # final dots tile
            pk, pv = prev
            for p in range(4):
                nc.tensor.matmul(
                    dots_t[p],
                    pv[:, ts(p, 128)],
                    pk[:, (p // 2) * 256 : (p // 2) * 256 + 256],
                    start=False,
                    stop=True,
                )
            for p in range(4):
                nc.vector.tensor_copy(
                    dots_sb[:, ts(p, 128)],
                    dots_t[p][:, (p % 2) * 128 : (p % 2) * 128 + 128],
                )
            nc.sync.dma_start(cc_in.ap(), dots_sb)
            nc.gpsimd.collective_compute(
                "AllReduce",
                ALU.add,
                replica_groups=[[0, 1], [2, 3], [4, 5], [6, 7]],
                ins=[cc_in.ap()],
                outs=[cc_out.ap()],
            )

            # x tail: iterations LAG..NT-1 overlap the AllReduce (no gpsimd).
            # The last tail group's DMAs are issued at j == NT-4, so the
            # AllReduce-result fetch + W2 slot in after that without
            # blocking the sync queue, and the first 8 output pairs are
            # emitted at the end of the tail so the PE chews them while
            # the DVE finishes the tail rotary.
            for j in range(NT - LAG, NT - 3):
                x_iter(j, zin, elem, stats, q_pp, use_gpsimd=False)

            # dots -> W2 = blkdiag(dotsT) @ WoT / n2 (reuses the freed
            # dots accumulation banks)
            nc.sync.dma_start(dots_rd, cc_out.ap())
            nc.vector.memset(blk, 0.0)
            for p in range(4):
                nc.vector.tensor_copy(
                    blk[0:64, p * 128 : p * 128 + 64],
                    dots_rd[0:64, p * 128 : p * 128 + 64],
                )
                nc.vector.tensor_copy(
                    blk[64:128, p * 128 + 64 : p * 128 + 128],
                    dots_rd[64:128, p * 128 + 64 : p * 128 + 128],
                )
            for p in range(4):
                w2_ps = dots_pp.tile([128, 256], F32, tag=f"dots{p}")
                nc.tensor.matmul(w2_ps, blk[:, ts(p, 128)], wo_t[p])
                nc.scalar.mul(w2_sb[:, ts(p, 256)], w2_ps, 1.0 / N1)

            for j in range(NT - 3, NT):
                x_iter(j, zin, elem, stats, q_pp, use_gpsimd=False)

            # output pairs 0..7 (row tiles 0..15): PSUM via kv/q tag reuse
            for t2 in range(NT // 4):
                ytag = "kv_ps" if t2 % 2 == 0 else "q_ps"
                ypool = kv_pp if t2 % 2 == 0 else q_pp
                y_ps = ypool.tile([128, 1024], F32, tag=ytag)
                for j in range(2):
                    t = 2 * t2 + j
                    for c in range(4):
                        nc.tensor.matmul(
                            y_ps[:, j * DIM : (j + 1) * DIM],
                            rotq_sb[c][:, ts(t, 128)],
                            w2_sb[:, ts(c, 256)],
                            start=(c == 0),
                            stop=(c == 3),
                        )
                y_sb = elem.tile([128, 2 * DIM], F32, tag="y_sb2")
                nc.vector.tensor_tensor(y_sb, y_ps[:, 0 : 2 * DIM], bo_bc, op=ALU.add)
                y_dst = bass.AP(
                    tensor=y_d.tensor,
                    offset=y_d.offset + t2 * 256 * DIM,
                    ap=[[DIM, 128], [128 * DIM, 2], [1, DIM]],
                )
                nc.sync.dma_start(y_dst, y_sb)

        # ---------------- phase Y: y = rotqT.T @ W2 + bo ----------------
        # two row-tiles per PSUM bank; one bias add + one DMA per pair
        with ExitStack() as py:
            outp = py.enter_context(tc.tile_pool(name="outp", bufs=3))
            y_pp = py.enter_context(tc.tile_pool(name="y_pp", bufs=2, space="PSUM"))

            for t2 in range(NT // 4, NT // 2):
                y_ps = y_pp.tile([128, 2 * DIM], F32, tag="y_ps")
                for j in range(2):
                    t = 2 * t2 + j
                    for c in range(4):
                        nc.tensor.matmul(
                            y_ps[:, j * DIM : (j + 1) * DIM],
                            rotq_sb[c][:, ts(t, 128)],
                            w2_sb[:, ts(c, 256)],
                            start=(c == 0),
                            stop=(c == 3),
                        )
                y_sb = outp.tile([128, 2 * DIM], F32, tag="y_sb")
                nc.vector.tensor_tensor(y_sb, y_ps, bo_bc, op=ALU.add)
                y_dst = bass.AP(
                    tensor=y_d.tensor,
                    offset=y_d.offset + t2 * 256 * DIM,
                    ap=[[DIM, 128], [128 * DIM, 2], [1, DIM]],
                )
                nc.sync.dma_start(y_dst, y_sb)

    nc.compile()
    _nc_cache[triv] = nc
    return nc


def _freqs(pos):
    """pos [rows, 2] -> f [rows, 32] = 16 x-freqs || 16 y-freqs."""
    rdim = DH // 2  # 32
    inv_freq = (
        1.0 / (10000.0 ** (np.arange(0, rdim, 2, dtype=np.float64) / rdim))
    ).astype(np.float64)  # [16]
    t = pos.astype(np.float64) * 64.0  # SCALE / MIN_FREQ
    return np.concatenate([t[:, 0:1] * inv_freq, t[:, 1:2] * inv_freq], axis=1)


# inner-dim helpers: d (0..63 within head) = a*32 + qd*16 + e
_d = np.arange(64)
_a = _d // 32
_qd = (_d % 32) // 16
_e = _d % 16
_freq_idx = _a * 16 + _e  # [64] -> col in [0,32)
_sign = np.where(_qd == 0, -1.0, 1.0)  # quad0: -sin, quad1: +sin
_swap = _d + 16 - 32 * _qd  # rotate-half partner within head


def _ktables(pos, k_gamma):
    """k-side row-major tables [rows, 128]: cos*gamma || sign*sin*gamma_swap."""
    f = _freqs(pos)  # [rows, 32]
    g = np.asarray(k_gamma, dtype=np.float64)
    cos = np.cos(f)[:, _freq_idx] * g[None, :]
    sin = np.sin(f)[:, _freq_idx] * _sign[None, :] * g[_swap][None, :]
    return np.ascontiguousarray(
        np.concatenate([cos, sin], axis=1).astype(np.float16)
    )


def _qtables(pos):
    """q-side partition-replicated tables [128, 2*rows]: cos || sign*sin."""
    f = _freqs(pos)  # [rows, 32]
    cosT = np.cos(f).T  # [32, rows]
    sinT = np.sin(f).T
    d128 = np.arange(128) % 64
    cos_rep = cosT[_freq_idx[d128], :]  # [128, rows]
    sin_rep = sinT[_freq_idx[d128], :] * _sign[d128][:, None]
    return np.ascontiguousarray(
        np.concatenate([cos_rep, sin_rep], axis=1).astype(np.float16)
    )


def _rbk_table(pos, k_beta):
    """rot(beta_k) [rows, 64] for the non-trivial-affine path."""
    f = _freqs(pos)
    b = np.asarray(k_beta, dtype=np.float64)
    return np.ascontiguousarray(
        (
            np.cos(f)[:, _freq_idx] * b[None, :]
            + np.sin(f)[:, _freq_idx] * _sign[None, :] * b[_swap][None, :]
        ).astype(np.float16)
    )


def _prepare(
    x, z, x_pos, z_pos, Wq, Wkv, k_gamma, k_beta, v_gamma, v_beta, Wo, bo
):
    """Host prep: returns (nc, in_maps) ready for run_bass_kernel_spmd."""
    xf = np.asarray(x, dtype=np.float32).reshape(B * N1, DIM)
    zf = np.asarray(z, dtype=np.float32).reshape(B * N1, DIM)
    xpf = np.asarray(x_pos).reshape(B * N1, 2)
    zpf = np.asarray(z_pos).reshape(B * N1, 2)

    triv = bool(
        np.all(np.asarray(k_gamma) == 1.0)
        and np.all(np.asarray(k_beta) == 0.0)
        and np.all(np.asarray(v_gamma) == 1.0)
        and np.all(np.asarray(v_beta) == 0.0)
    )

    wqT = np.asarray(Wq, dtype=np.float64).T  # [256, 512]
    j = np.arange(INNER)
    swap_full = (j // 64) * 64 + _swap[j % 64]
    wqrhT = np.ascontiguousarray(wqT[:, swap_full].astype(np.float16))
    wqT = np.ascontiguousarray(wqT.astype(np.float16))

    # Wkv: center per 64-col head block (exact mean removal), fold gamma_v
    # into the v half.
    wkvT = np.asarray(Wkv, dtype=np.float64).T  # [256, 1024]
    wkv_c = wkvT.reshape(DIM, 16, DH)
    wkv_c = wkv_c - wkv_c.mean(axis=2, keepdims=True)
    if not triv:
        gv = np.asarray(v_gamma, dtype=np.float64)
        wkv_c[:, 8:16, :] = wkv_c[:, 8:16, :] * gv[None, None, :]
    wkvT = np.ascontiguousarray(wkv_c.reshape(DIM, 2 * INNER).astype(np.float16))

    woT = np.ascontiguousarray(np.asarray(Wo).T.astype(np.float16))  # [512, 256]
    bo_r = np.ascontiguousarray(np.tile(np.asarray(bo).reshape(1, DIM), (1, 2))).astype(
        np.float32
    )

    kg = np.asarray(k_gamma, dtype=np.float64) if not triv else np.ones(DH)

    nc = build_nc(triv)

    in_maps = []
    for c in range(NCORES):
        lo, hi = c * ROWS, (c + 1) * ROWS
        m = {
            "zt": np.ascontiguousarray(zf[lo:hi].T.astype(np.float16)),
            "xt": np.ascontiguousarray(xf[lo:hi].T.astype(np.float16)),
            "wq": wqT,
            "wqrh": wqrhT,
            "wkv": wkvT,
            "wo": woT,
            "bo": bo_r,
            "ktab": _ktables(zpf[lo:hi], kg),
            "qtab": _qtables(xpf[lo:hi]),
        }
        if not triv:
            m["rbk"] = _rbk_table(zpf[lo:hi], k_beta)
            m["bv"] = np.ascontiguousarray(
                np.tile(np.asarray(v_beta), H)[None, :]
            ).astype(np.float32)
        in_maps.append(m)
    return nc, in_maps


def kernel(**inputs):
    nc, in_maps = _prepare(**inputs)
    res = run_bass_kernel_spmd(nc, in_maps, list(range(NCORES)))
    y = np.concatenate([res.results[c]["y"] for c in range(NCORES)], axis=0)
    return y.reshape(B, N1, DIM).astype(np.float32)
